# revision 2
# baseline (speedup 1.0000x reference)
"""Trainium2 Bass kernel for 2-layer GraphSAGE (mean aggregator), 8 NeuronCores.

Sharding: layer 0 dst-sharded (feat replicated, per-core edge buckets, local
dma_gather + one-hot matmul segment-sum); layer 1 src-sharded with a single
ReduceScatter of partial message sums; final matmuls on the dst owner.
"""
import os
import sys

sys.path.insert(0, "/opt/trn_rl_repo")

import numpy as np

N_SRC0, N_DST0 = 200000, 40000
N_SRC1, N_DST1 = 40000, 8000
E0, E1 = 1000000, 80000
D_IN, D_HID, D_OUT = 256, 512, 256
C = 8                      # cores
RANGE = 32768              # int16 gather index range
NR0 = (N_SRC0 + RANGE - 1) // RANGE   # 7 src ranges, layer 0
PAD_DST1 = 8064            # 63 * 128
SH1 = PAD_DST1 // C        # 1008 rows per core after ReduceScatter
NB0 = 40                   # local dst blocks of 128 (5120 padded local rows)
NLOC = 5000                # real local dst rows, layer 0
NB1 = PAD_DST1 // 128      # 63 global dst blocks, layer 1
NBF = 8                    # final blocks over 1008 rows (7*128 + 112)

F32 = None  # set after import
LAST_RESULT = None  # BassKernelResults of the most recent run (for test.py)


def _round16(x):
    return (x + 15) // 16 * 16


def _wrap_idx(slots):
    """int16 slot list (len % 16 == 0) -> [128, len//16] wrapped/replicated."""
    n = len(slots)
    w = slots.reshape(n // 16, 16).T            # [16, n//16]
    return np.tile(w, (8, 1)).astype(np.int16)  # [128, n//16]


def _host_prep(feat, Wself0, Wneigh0, b0, Wself1, Wneigh1, b1,
               edge_src0, edge_dst0, edge_src1, edge_dst1):
    src0 = np.asarray(edge_src0).astype(np.int64)
    dst0 = np.asarray(edge_dst0).astype(np.int64)
    src1 = np.asarray(edge_src1).astype(np.int64)
    dst1 = np.asarray(edge_dst1).astype(np.int64)
    feat = np.asarray(feat, dtype=np.float32)

    # ---- ownership of layer-0 dst rows (also layer-1 src rows) ----
    s_core = np.full(C, SH1, np.int64)
    s_core[C - 1] = N_DST1 - SH1 * (C - 1)          # 944
    remB = NLOC - s_core
    baseB = N_DST1 + np.concatenate(([0], np.cumsum(remB)[:-1]))
    own = np.empty(N_DST0, np.int64)
    loc = np.empty(N_DST0, np.int64)
    local2global = []
    for c in range(C):
        ga = np.arange(SH1 * c, SH1 * c + s_core[c])
        gb = np.arange(baseB[c], baseB[c] + remB[c])
        own[ga] = c
        loc[ga] = np.arange(s_core[c])
        own[gb] = c
        loc[gb] = s_core[c] + np.arange(remB[c])
        local2global.append(np.concatenate([ga, gb]))

    # ---- layer 0 buckets: (core, local block, src range) ----
    ec0 = own[dst0]
    lb0 = loc[dst0] // 128
    lp0 = loc[dst0] % 128
    rk0 = src0 // RANGE
    sl0 = (src0 - rk0 * RANGE).astype(np.int64)

    key0 = (ec0 * NB0 + lb0) * NR0 + rk0
    cnt0 = np.bincount(key0, minlength=C * NB0 * NR0).reshape(C, NB0, NR0)
    Q0 = _round16(cnt0.max(axis=0))                 # [NB0, NR0]
    Q0[cnt0.max(axis=0) == 0] = 0
    CK0 = (Q0 + 127) // 128                         # chunks per bucket
    slot_off0 = np.zeros((NB0, NR0), np.int64)      # slot offset per bucket
    chunk_off0 = np.zeros((NB0, NR0), np.int64)
    so = 0
    co = 0
    for b in range(NB0):
        for k in range(NR0):
            slot_off0[b, k] = so
            chunk_off0[b, k] = co
            so += Q0[b, k]
            co += CK0[b, k]
    TOT0 = int(so)
    NC0 = int(co)

    order0 = np.argsort(key0, kind="stable")
    s_src = sl0[order0]
    s_pos = lp0[order0]
    s_key = key0[order0]
    seg_start0 = np.searchsorted(s_key, np.arange(C * NB0 * NR0))
    seg_end0 = np.searchsorted(s_key, np.arange(C * NB0 * NR0) + 1)

    # ---- layer 1 buckets: (owner core of src1, global dst block) ----
    ec1 = own[src1]
    sl1 = loc[src1]
    gb1 = dst1 // 128
    gp1 = dst1 % 128
    key1 = ec1 * NB1 + gb1
    cnt1 = np.bincount(key1, minlength=C * NB1).reshape(C, NB1)
    Q1 = _round16(cnt1.max(axis=0))
    Q1[cnt1.max(axis=0) == 0] = 0
    CK1 = (Q1 + 127) // 128
    slot_off1 = np.zeros(NB1, np.int64)
    chunk_off1 = np.zeros(NB1, np.int64)
    so = 0
    co = 0
    for b in range(NB1):
        slot_off1[b] = so
        chunk_off1[b] = co
        so += Q1[b]
        co += CK1[b]
    TOT1 = int(so)
    NC1 = int(co)

    order1 = np.argsort(key1, kind="stable")
    t_src = sl1[order1]
    t_pos = gp1[order1]
    t_key = key1[order1]
    seg_start1 = np.searchsorted(t_key, np.arange(C * NB1))
    seg_end1 = np.searchsorted(t_key, np.arange(C * NB1) + 1)

    # ---- degrees ----
    deg0 = np.bincount(dst0, minlength=N_DST0).astype(np.float32)
    deg1 = np.bincount(dst1, minlength=N_DST1).astype(np.float32)

    # ---- weight layouts (replicated) ----
    w0s = np.ascontiguousarray(
        np.asarray(Wself0, np.float32).reshape(2, 128, D_HID).transpose(1, 0, 2)
    ).reshape(128, 2 * D_HID)
    w0n = np.ascontiguousarray(
        np.asarray(Wneigh0, np.float32).reshape(2, 128, D_HID).transpose(1, 0, 2)
    ).reshape(128, 2 * D_HID)
    w1s = np.ascontiguousarray(
        np.asarray(Wself1, np.float32).reshape(4, 128, D_OUT).transpose(1, 0, 2)
    ).reshape(128, 4 * D_OUT)
    w1n = np.ascontiguousarray(
        np.asarray(Wneigh1, np.float32).reshape(4, 128, D_OUT).transpose(1, 0, 2)
    ).reshape(128, 4 * D_OUT)
    b0r = np.asarray(b0, np.float32).reshape(1, D_HID)
    b1r = np.asarray(b1, np.float32).reshape(1, D_OUT)
    ii = np.zeros((128, 256), np.float32)
    ii[:, :128] = np.arange(128, dtype=np.float32)[None, :]
    ii[:, 128:] = np.eye(128, dtype=np.float32)

    # ---- per-core data ----
    in_maps = []
    for c in range(C):
        slots_src0 = np.zeros(TOT0, np.int64)
        slots_pos0 = np.full(NC0 * 128, -1.0, np.float32)
        for b in range(NB0):
            for k in range(NR0):
                if Q0[b, k] == 0:
                    continue
                kk = (c * NB0 + b) * NR0 + k
                a, e = seg_start0[kk], seg_end0[kk]
                n = e - a
                off = slot_off0[b, k]
                slots_src0[off:off + n] = s_src[a:e]
                coff = chunk_off0[b, k] * 128
                slots_pos0[coff:coff + n] = s_pos[a:e].astype(np.float32)
        idx0 = _wrap_idx(slots_src0.astype(np.int16))
        dl0 = np.ascontiguousarray(
            slots_pos0.reshape(NC0, 128).T)          # [128, NC0]

        slots_src1 = np.zeros(TOT1, np.int64)
        slots_pos1 = np.full(NC1 * 128, -1.0, np.float32)
        for b in range(NB1):
            if Q1[b] == 0:
                continue
            kk = c * NB1 + b
            a, e = seg_start1[kk], seg_end1[kk]
            n = e - a
            off = slot_off1[b]
            slots_src1[off:off + n] = t_src[a:e]
            coff = chunk_off1[b] * 128
            slots_pos1[coff:coff + n] = t_pos[a:e].astype(np.float32)
        idx1 = _wrap_idx(slots_src1.astype(np.int16))
        dl1 = np.ascontiguousarray(slots_pos1.reshape(NC1, 128).T)

        degloc = np.ones(NB0 * 128, np.float32)
        degloc[:NLOC] = deg0[local2global[c]]
        inv0 = np.ascontiguousarray(
            (1.0 / np.maximum(degloc, 1.0)).reshape(NB0, 128).T)

        deg1loc = np.ones(NBF * 128, np.float32)
        gl = SH1 * c + np.arange(SH1)
        valid = gl < N_DST1
        deg1loc[:SH1][valid] = deg1[gl[valid]]
        inv1 = np.ascontiguousarray(
            (1.0 / np.maximum(deg1loc, 1.0)).reshape(NBF, 128).T)

        fself = np.zeros((NB0 * 128, D_IN), np.float32)
        fself[:NLOC] = feat[local2global[c]]

        in_maps.append({
            "feat": feat, "fself": fself,
            "idx0": idx0, "dl0": dl0, "inv0": inv0,
            "idx1": idx1, "dl1": dl1, "inv1": inv1,
            "w0s": w0s, "w0n": w0n, "b0r": b0r,
            "w1s": w1s, "w1n": w1n, "b1r": b1r, "ii": ii,
        })

    meta = dict(Q0=Q0, CK0=CK0, slot_off0=slot_off0, chunk_off0=chunk_off0,
                TOT0=TOT0, NC0=NC0,
                Q1=Q1, CK1=CK1, slot_off1=slot_off1, chunk_off1=chunk_off1,
                TOT1=TOT1, NC1=NC1, s_core=s_core)
    return in_maps, meta


def _build_program(meta):
    import concourse.bass as bass
    import concourse.mybir as mybir
    import concourse.tile as tile
    from concourse import bacc

    f32 = mybir.dt.float32
    i16 = mybir.dt.int16
    Q0, CK0 = meta["Q0"], meta["CK0"]
    slot_off0, chunk_off0 = meta["slot_off0"], meta["chunk_off0"]
    TOT0, NC0 = meta["TOT0"], meta["NC0"]
    Q1, CK1 = meta["Q1"], meta["CK1"]
    slot_off1, chunk_off1 = meta["slot_off1"], meta["chunk_off1"]
    TOT1, NC1 = meta["TOT1"], meta["NC1"]

    nc = bacc.Bacc("TRN2", target_bir_lowering=False, debug=False,
                   enable_asserts=True, num_devices=C)
    t_feat = nc.dram_tensor("feat", [N_SRC0, D_IN], f32, kind="ExternalInput")
    t_fself = nc.dram_tensor("fself", [NB0 * 128, D_IN], f32, kind="ExternalInput")
    t_idx0 = nc.dram_tensor("idx0", [128, TOT0 // 16], i16, kind="ExternalInput")
    t_dl0 = nc.dram_tensor("dl0", [128, NC0], f32, kind="ExternalInput")
    t_inv0 = nc.dram_tensor("inv0", [128, NB0], f32, kind="ExternalInput")
    t_idx1 = nc.dram_tensor("idx1", [128, TOT1 // 16], i16, kind="ExternalInput")
    t_dl1 = nc.dram_tensor("dl1", [128, NC1], f32, kind="ExternalInput")
    t_inv1 = nc.dram_tensor("inv1", [128, NBF], f32, kind="ExternalInput")
    t_w0s = nc.dram_tensor("w0s", [128, 2 * D_HID], f32, kind="ExternalInput")
    t_w0n = nc.dram_tensor("w0n", [128, 2 * D_HID], f32, kind="ExternalInput")
    t_b0 = nc.dram_tensor("b0r", [1, D_HID], f32, kind="ExternalInput")
    t_w1s = nc.dram_tensor("w1s", [128, 4 * D_OUT], f32, kind="ExternalInput")
    t_w1n = nc.dram_tensor("w1n", [128, 4 * D_OUT], f32, kind="ExternalInput")
    t_b1 = nc.dram_tensor("b1r", [1, D_OUT], f32, kind="ExternalInput")
    t_ii = nc.dram_tensor("ii", [128, 256], f32, kind="ExternalInput")
    t_out = nc.dram_tensor("out", [SH1, D_OUT], f32, kind="ExternalOutput")
    t_hmy = nc.dram_tensor("hmy", [NB0 * 128, D_HID], f32)
    t_partial = nc.dram_tensor("partial", [PAD_DST1, D_HID], f32)
    t_rs = nc.dram_tensor("rsout", [SH1, D_HID], f32)

    eq = mybir.AluOpType.is_equal
    Relu = mybir.ActivationFunctionType.Relu

    with tile.TileContext(nc) as tc:
        with tc.tile_pool(name="const", bufs=1) as cp:
            idx0_t = cp.tile([128, TOT0 // 16], i16)
            nc.sync.dma_start(idx0_t[:], t_idx0[:, :])
            dl0_t = cp.tile([128, NC0], f32)
            nc.sync.dma_start(dl0_t[:], t_dl0[:, :])
            inv0_t = cp.tile([128, NB0], f32)
            nc.sync.dma_start(inv0_t[:], t_inv0[:, :])
            idx1_t = cp.tile([128, TOT1 // 16], i16)
            nc.sync.dma_start(idx1_t[:], t_idx1[:, :])
            dl1_t = cp.tile([128, NC1], f32)
            nc.sync.dma_start(dl1_t[:], t_dl1[:, :])
            inv1_t = cp.tile([128, NBF], f32)
            nc.sync.dma_start(inv1_t[:], t_inv1[:, :])
            w0s_t = cp.tile([128, 2 * D_HID], f32)
            nc.sync.dma_start(w0s_t[:], t_w0s[:, :])
            w0n_t = cp.tile([128, 2 * D_HID], f32)
            nc.sync.dma_start(w0n_t[:], t_w0n[:, :])
            b0_t = cp.tile([1, D_HID], f32)
            nc.sync.dma_start(b0_t[:], t_b0[:, :])
            w1s_t = cp.tile([128, 4 * D_OUT], f32)
            nc.sync.dma_start(w1s_t[:], t_w1s[:, :])
            w1n_t = cp.tile([128, 4 * D_OUT], f32)
            nc.sync.dma_start(w1n_t[:], t_w1n[:, :])
            b1_t = cp.tile([1, D_OUT], f32)
            nc.sync.dma_start(b1_t[:], t_b1[:, :])
            ii_t = cp.tile([128, 256], f32)
            nc.sync.dma_start(ii_t[:], t_ii[:, :])
            ones_t = cp.tile([1, 128], f32)
            nc.vector.memset(ones_t[:], 1.0)
            iota_t = ii_t[:, 0:128]
            ident_t = ii_t[:, 128:256]

            # ================= Layer 0 =================
            with tc.tile_pool(name="g0", bufs=3) as g0p, \
                 tc.tile_pool(name="s0", bufs=4) as s0p, \
                 tc.tile_pool(name="mm0", bufs=2) as mm0p, \
                 tc.tile_pool(name="xs0", bufs=2) as xs0p, \
                 tc.tile_pool(name="xt0", bufs=2) as xt0p, \
                 tc.tile_pool(name="h0", bufs=3) as h0p, \
                 tc.tile_pool(name="pmsg", bufs=2, space="PSUM") as pmsgp, \
                 tc.tile_pool(name="pout", bufs=2, space="PSUM") as poutp, \
                 tc.tile_pool(name="pt", bufs=2, space="PSUM") as ptp:
                for b in range(NB0):
                    nchunks = int(CK0[b].sum())
                    psum_msg = pmsgp.tile([128, D_IN], f32)
                    ci = 0
                    for k in range(NR0):
                        Q = int(Q0[b, k])
                        if Q == 0:
                            continue
                        CK = int(CK0[b, k])
                        rbase = k * RANGE
                        rsz = min(RANGE, N_SRC0 - rbase)
                        g = g0p.tile([128, CK, D_IN], f32, tag="g0")
                        c0 = int(slot_off0[b, k]) // 16
                        nc.gpsimd.dma_gather(
                            g[:, :, :], t_feat[rbase:rbase + rsz, :],
                            idx0_t[:, c0:c0 + Q // 16],
                            num_idxs=Q, num_idxs_reg=Q, elem_size=D_IN)
                        for j in range(CK):
                            q = int(chunk_off0[b, k]) + j
                            S = s0p.tile([128, 128], f32, tag="s0")
                            nc.vector.tensor_tensor(
                                out=S[:],
                                in0=dl0_t[:, q:q + 1].to_broadcast([128, 128]),
                                in1=iota_t, op=eq)
                            nc.tensor.matmul(
                                psum_msg[:], lhsT=S[:], rhs=g[:, j, :],
                                start=(ci == 0), stop=(ci == nchunks - 1))
                            ci += 1
                    msgm = mm0p.tile([128, D_IN], f32)
                    nc.vector.tensor_scalar_mul(
                        msgm[:], psum_msg[:], inv0_t[:, b:b + 1])
                    xs = xs0p.tile([128, D_IN], f32)
                    nc.sync.dma_start(xs[:], t_fself[b * 128:(b + 1) * 128, :])
                    xts = xt0p.tile([128, D_IN], f32, tag="xts")
                    xtm = xt0p.tile([128, D_IN], f32, tag="xtm")
                    for src_t, dst_t in ((xs, xts), (msgm, xtm)):
                        for kk in range(2):
                            pt = ptp.tile([128, 128], f32)
                            nc.tensor.transpose(
                                pt[:], src_t[:, kk * 128:(kk + 1) * 128],
                                ident_t)
                            nc.vector.tensor_copy(
                                dst_t[:, kk * 128:(kk + 1) * 128], pt[:])
                    pout = poutp.tile([128, D_HID], f32)
                    nc.tensor.matmul(pout[:], lhsT=xts[:, 0:128],
                                     rhs=w0s_t[:, 0:D_HID],
                                     start=True, stop=False)
                    nc.tensor.matmul(pout[:], lhsT=xts[:, 128:256],
                                     rhs=w0s_t[:, D_HID:2 * D_HID],
                                     start=False, stop=False)
                    nc.tensor.matmul(pout[:], lhsT=xtm[:, 0:128],
                                     rhs=w0n_t[:, 0:D_HID],
                                     start=False, stop=False)
                    nc.tensor.matmul(pout[:], lhsT=xtm[:, 128:256],
                                     rhs=w0n_t[:, D_HID:2 * D_HID],
                                     start=False, stop=False)
                    nc.tensor.matmul(pout[:], lhsT=ones_t[:1, :],
                                     rhs=b0_t[:1, :], start=False, stop=True)
                    h = h0p.tile([128, D_HID], f32)
                    nc.scalar.activation(h[:], pout[:], Relu)
                    nc.sync.dma_start(t_hmy[b * 128:(b + 1) * 128, :], h[:])

            # ================= Layer 1 partial sums =================
            with tc.tile_pool(name="g1", bufs=3) as g1p, \
                 tc.tile_pool(name="s1", bufs=4) as s1p, \
                 tc.tile_pool(name="pp1", bufs=3) as pp1p, \
                 tc.tile_pool(name="pm1", bufs=2, space="PSUM") as pm1p:
                for b in range(NB1):
                    Q = int(Q1[b])
                    if Q == 0:
                        continue
                    CK = int(CK1[b])
                    psum1 = pm1p.tile([128, D_HID], f32)
                    g = g1p.tile([128, CK, D_HID], f32, tag="g1")
                    c0 = int(slot_off1[b]) // 16
                    nc.gpsimd.dma_gather(
                        g[:, :, :], t_hmy[:, :], idx1_t[:, c0:c0 + Q // 16],
                        num_idxs=Q, num_idxs_reg=Q, elem_size=D_HID)
                    for j in range(CK):
                        q = int(chunk_off1[b]) + j
                        S = s1p.tile([128, 128], f32, tag="s1")
                        nc.vector.tensor_tensor(
                            out=S[:],
                            in0=dl1_t[:, q:q + 1].to_broadcast([128, 128]),
                            in1=iota_t, op=eq)
                        nc.tensor.matmul(
                            psum1[:], lhsT=S[:], rhs=g[:, j, :],
                            start=(j == 0), stop=(j == CK - 1))
                    part = pp1p.tile([128, D_HID], f32)
                    nc.vector.tensor_copy(part[:], psum1[:])
                    nc.sync.dma_start(
                        t_partial[b * 128:(b + 1) * 128, :], part[:])

            # ================= ReduceScatter =================
            nc.gpsimd.collective_compute(
                "ReduceScatter", mybir.AluOpType.add,
                replica_groups=[list(range(C))],
                ins=[t_partial.ap().opt()], outs=[t_rs.ap().opt()])

            # ================= Final layer-1 matmuls =================
            with tc.tile_pool(name="mf", bufs=2) as mfp, \
                 tc.tile_pool(name="hsf", bufs=2) as hsfp, \
                 tc.tile_pool(name="xtf", bufs=2) as xtfp, \
                 tc.tile_pool(name="of", bufs=2) as ofp, \
                 tc.tile_pool(name="pf", bufs=2, space="PSUM") as pfp, \
                 tc.tile_pool(name="ptf", bufs=2, space="PSUM") as ptfp:
                for fb in range(NBF):
                    nr = 128 if fb < NBF - 1 else SH1 - 128 * (NBF - 1)
                    m1 = mfp.tile([128, D_HID], f32, tag="m1")
                    nc.sync.dma_start(
                        m1[:nr, :], t_rs[fb * 128:fb * 128 + nr, :])
                    mm = mfp.tile([128, D_HID], f32, tag="mm")
                    nc.vector.tensor_scalar_mul(
                        mm[:nr, :], m1[:nr, :], inv1_t[:nr, fb:fb + 1])
                    hs = hsfp.tile([128, D_HID], f32)
                    nc.sync.dma_start(
                        hs[:nr, :], t_hmy[fb * 128:fb * 128 + nr, :])
                    xth = xtfp.tile([128, 4, 128], f32, tag="xth")
                    xtm1 = xtfp.tile([128, 4, 128], f32, tag="xtm1")
                    for src_t, dst_t in ((hs, xth), (mm, xtm1)):
                        for kk in range(4):
                            pt = ptfp.tile([128, 128], f32)
                            nc.tensor.transpose(
                                pt[:, :nr],
                                src_t[:nr, kk * 128:(kk + 1) * 128],
                                ident_t[:nr, :nr])
                            nc.vector.tensor_copy(
                                dst_t[:, kk, :nr], pt[:, :nr])
                    pf = pfp.tile([128, D_OUT], f32)
                    for kk in range(4):
                        nc.tensor.matmul(
                            pf[:nr, :], lhsT=xth[:, kk, :nr],
                            rhs=w1s_t[:, kk * D_OUT:(kk + 1) * D_OUT],
                            start=(kk == 0), stop=False)
                    for kk in range(4):
                        nc.tensor.matmul(
                            pf[:nr, :], lhsT=xtm1[:, kk, :nr],
                            rhs=w1n_t[:, kk * D_OUT:(kk + 1) * D_OUT],
                            start=False, stop=False)
                    nc.tensor.matmul(pf[:nr, :], lhsT=ones_t[:1, :nr],
                                     rhs=b1_t[:1, :], start=False, stop=True)
                    ot = ofp.tile([128, D_OUT], f32)
                    nc.vector.tensor_copy(ot[:nr, :], pf[:nr, :])
                    nc.sync.dma_start(
                        t_out[fb * 128:fb * 128 + nr, :], ot[:nr, :])

    nc.compile()
    return nc


def _ensure_axon_hook():
    """Provide antenv.axon_hooks (missing in this image) so trace=True can
    capture NTFF profiles through the axon tunnel."""
    import types

    try:
        from antenv.axon_hooks import get_axon_ntff_profile_hook  # noqa: F401
        return
    except ImportError:
        pass
    mod = types.ModuleType("antenv.axon_hooks")
    _h = [None]
    mod.set_axon_ntff_profile_hook = lambda h: _h.__setitem__(0, h)
    mod.get_axon_ntff_profile_hook = lambda: _h[0]
    sys.modules["antenv.axon_hooks"] = mod
    import antenv

    antenv.axon_hooks = mod
    try:
        from trn_agent_boot.trn_boot import _ntff_profile_via_ctypes

        hook = _ntff_profile_via_ctypes("/opt/axon/libaxon_pjrt.so")
        if hook is not None:
            mod.set_axon_ntff_profile_hook(hook)
    except Exception:
        pass


def kernel(feat, Wself0, Wneigh0, b0, Wself1, Wneigh1, b1,
           edge_src0, edge_dst0, edge_src1, edge_dst1):
    global LAST_RESULT
    from concourse.bass_utils import run_bass_kernel_spmd

    if int(os.environ.get("KERNEL_TRACE", "0")):
        _ensure_axon_hook()

    in_maps, meta = _host_prep(feat, Wself0, Wneigh0, b0,
                               Wself1, Wneigh1, b1,
                               edge_src0, edge_dst0, edge_src1, edge_dst1)
    nc = _build_program(meta)
    res = run_bass_kernel_spmd(nc, in_maps, core_ids=list(range(C)),
                               trace=bool(int(os.environ.get("KERNEL_TRACE", "0"))))
    LAST_RESULT = res
    s_core = meta["s_core"]
    out = np.empty((N_DST1, D_OUT), np.float32)
    for c in range(C):
        out[SH1 * c:SH1 * c + s_core[c]] = res.results[c]["out"][:s_core[c]]
    return out


# revision 4
# speedup vs baseline: 1.0867x; 1.0867x over previous
"""Trainium2 Bass kernel for 2-layer GraphSAGE (mean aggregator), 8 NeuronCores.

Sharding: layer 0 dst-sharded (feat replicated, per-core edge buckets, local
dma_gather + one-hot matmul segment-sum); layer 1 src-sharded with a single
ReduceScatter of partial message sums; final matmuls on the dst owner.
"""
import os
import sys

sys.path.insert(0, "/opt/trn_rl_repo")

import numpy as np

N_SRC0, N_DST0 = 200000, 40000
N_SRC1, N_DST1 = 40000, 8000
E0, E1 = 1000000, 80000
D_IN, D_HID, D_OUT = 256, 512, 256
C = 8                      # cores
RANGE = 32768              # int16 gather index range
NR0 = (N_SRC0 + RANGE - 1) // RANGE   # 7 src ranges, layer 0
PAD_DST1 = 8064            # 63 * 128
SH1 = PAD_DST1 // C        # 1008 rows per core after ReduceScatter
NB0 = 40                   # local dst blocks of 128 (5120 padded local rows)
NLOC = 5000                # real local dst rows, layer 0
NB1 = PAD_DST1 // 128      # 63 global dst blocks, layer 1
NBF = 8                    # final blocks over 1008 rows (7*128 + 112)

F32 = None  # set after import
LAST_RESULT = None  # BassKernelResults of the most recent run (for test.py)


def _round16(x):
    return (x + 15) // 16 * 16


def _wrap_idx(slots):
    """int16 slot list (len % 16 == 0) -> [128, len//16] wrapped/replicated."""
    n = len(slots)
    w = slots.reshape(n // 16, 16).T            # [16, n//16]
    return np.tile(w, (8, 1)).astype(np.int16)  # [128, n//16]


def _host_prep(feat, Wself0, Wneigh0, b0, Wself1, Wneigh1, b1,
               edge_src0, edge_dst0, edge_src1, edge_dst1):
    src0 = np.asarray(edge_src0).astype(np.int64)
    dst0 = np.asarray(edge_dst0).astype(np.int64)
    src1 = np.asarray(edge_src1).astype(np.int64)
    dst1 = np.asarray(edge_dst1).astype(np.int64)
    feat = np.asarray(feat, dtype=np.float32)

    # ---- ownership of layer-0 dst rows (also layer-1 src rows) ----
    s_core = np.full(C, SH1, np.int64)
    s_core[C - 1] = N_DST1 - SH1 * (C - 1)          # 944
    remB = NLOC - s_core
    baseB = N_DST1 + np.concatenate(([0], np.cumsum(remB)[:-1]))
    deg0_i = np.bincount(dst0, minlength=N_DST0)
    own = np.empty(N_DST0, np.int64)
    loc = np.empty(N_DST0, np.int64)
    local2global = []
    for c in range(C):
        ga = np.arange(SH1 * c, SH1 * c + s_core[c])
        gb = np.arange(baseB[c], baseB[c] + remB[c])
        # LPT-balance the non-pinned rows (locals >= s_c) across blocks so
        # per-block edge counts equalize -> less cross-core bucket padding.
        import heapq
        nA = int(s_core[c])
        cap = np.zeros(NB0, np.int64)
        tot = np.zeros(NB0, np.int64)
        for b in range(NB0):
            lo, hi = b * 128, min((b + 1) * 128, NLOC)
            if hi <= lo:
                continue
            pinned = max(0, min(hi, nA) - lo)
            cap[b] = (hi - lo) - pinned
            if pinned:
                tot[b] = deg0_i[ga[lo:lo + pinned]].sum()
        order = gb[np.argsort(-deg0_i[gb], kind="stable")]
        heap = [(int(tot[b]), b) for b in range(NB0) if cap[b] > 0]
        heapq.heapify(heap)
        assign = [[] for _ in range(NB0)]
        for g in order:
            t, b = heapq.heappop(heap)
            assign[b].append(g)
            cap[b] -= 1
            t += int(deg0_i[g])
            if cap[b] > 0:
                heapq.heappush(heap, (t, b))
        l2g = np.empty(NLOC, np.int64)
        l2g[:nA] = ga
        pos = nA
        for b in range(NB0):
            if assign[b]:
                l2g[pos:pos + len(assign[b])] = assign[b]
                pos += len(assign[b])
        assert pos == NLOC
        own[l2g] = c
        loc[l2g] = np.arange(NLOC)
        local2global.append(l2g)

    # ---- layer 0 buckets: (core, local block, src range) ----
    ec0 = own[dst0]
    lb0 = loc[dst0] // 128
    lp0 = loc[dst0] % 128
    rk0 = src0 // RANGE
    sl0 = (src0 - rk0 * RANGE).astype(np.int64)

    key0 = (ec0 * NB0 + lb0) * NR0 + rk0
    cnt0 = np.bincount(key0, minlength=C * NB0 * NR0).reshape(C, NB0, NR0)
    Q0 = _round16(cnt0.max(axis=0))                 # [NB0, NR0]
    Q0[cnt0.max(axis=0) == 0] = 0
    CK0 = (Q0 + 127) // 128                         # chunks per bucket
    slot_off0 = np.zeros((NB0, NR0), np.int64)      # slot offset per bucket
    chunk_off0 = np.zeros((NB0, NR0), np.int64)
    so = 0
    co = 0
    for b in range(NB0):
        for k in range(NR0):
            slot_off0[b, k] = so
            chunk_off0[b, k] = co
            so += Q0[b, k]
            co += CK0[b, k]
    TOT0 = int(so)
    NC0 = int(co)

    order0 = np.argsort(key0, kind="stable")
    s_src = sl0[order0]
    s_pos = lp0[order0]
    s_key = key0[order0]
    seg_start0 = np.searchsorted(s_key, np.arange(C * NB0 * NR0))
    seg_end0 = np.searchsorted(s_key, np.arange(C * NB0 * NR0) + 1)

    # ---- layer 1 buckets: (owner core of src1, global dst block) ----
    ec1 = own[src1]
    sl1 = loc[src1]
    gb1 = dst1 // 128
    gp1 = dst1 % 128
    key1 = ec1 * NB1 + gb1
    cnt1 = np.bincount(key1, minlength=C * NB1).reshape(C, NB1)
    Q1 = _round16(cnt1.max(axis=0))
    Q1[cnt1.max(axis=0) == 0] = 0
    CK1 = (Q1 + 127) // 128
    slot_off1 = np.zeros(NB1, np.int64)
    chunk_off1 = np.zeros(NB1, np.int64)
    so = 0
    co = 0
    for b in range(NB1):
        slot_off1[b] = so
        chunk_off1[b] = co
        so += Q1[b]
        co += CK1[b]
    TOT1 = int(so)
    NC1 = int(co)

    order1 = np.argsort(key1, kind="stable")
    t_src = sl1[order1]
    t_pos = gp1[order1]
    t_key = key1[order1]
    seg_start1 = np.searchsorted(t_key, np.arange(C * NB1))
    seg_end1 = np.searchsorted(t_key, np.arange(C * NB1) + 1)

    # ---- degrees ----
    deg0 = np.bincount(dst0, minlength=N_DST0).astype(np.float32)
    deg1 = np.bincount(dst1, minlength=N_DST1).astype(np.float32)

    # ---- weight layouts (replicated) ----
    w0s = np.ascontiguousarray(
        np.asarray(Wself0, np.float32).reshape(2, 128, D_HID).transpose(1, 0, 2)
    ).reshape(128, 2 * D_HID)
    w0n = np.ascontiguousarray(
        np.asarray(Wneigh0, np.float32).reshape(2, 128, D_HID).transpose(1, 0, 2)
    ).reshape(128, 2 * D_HID)
    w1s = np.ascontiguousarray(
        np.asarray(Wself1, np.float32).reshape(4, 128, D_OUT).transpose(1, 0, 2)
    ).reshape(128, 4 * D_OUT)
    w1n = np.ascontiguousarray(
        np.asarray(Wneigh1, np.float32).reshape(4, 128, D_OUT).transpose(1, 0, 2)
    ).reshape(128, 4 * D_OUT)
    b0r = np.asarray(b0, np.float32).reshape(1, D_HID)
    b1r = np.asarray(b1, np.float32).reshape(1, D_OUT)
    ii = np.zeros((128, 256), np.float32)
    ii[:, :128] = np.arange(128, dtype=np.float32)[None, :]
    ii[:, 128:] = np.eye(128, dtype=np.float32)

    # ---- per-core data ----
    in_maps = []
    for c in range(C):
        slots_src0 = np.zeros(TOT0, np.int64)
        slots_pos0 = np.full(NC0 * 128, -1.0, np.float32)
        for b in range(NB0):
            for k in range(NR0):
                if Q0[b, k] == 0:
                    continue
                kk = (c * NB0 + b) * NR0 + k
                a, e = seg_start0[kk], seg_end0[kk]
                n = e - a
                off = slot_off0[b, k]
                slots_src0[off:off + n] = s_src[a:e]
                coff = chunk_off0[b, k] * 128
                slots_pos0[coff:coff + n] = s_pos[a:e].astype(np.float32)
        idx0 = _wrap_idx(slots_src0.astype(np.int16))
        dl0 = np.ascontiguousarray(
            slots_pos0.reshape(NC0, 128).T)          # [128, NC0]

        slots_src1 = np.zeros(TOT1, np.int64)
        slots_pos1 = np.full(NC1 * 128, -1.0, np.float32)
        for b in range(NB1):
            if Q1[b] == 0:
                continue
            kk = c * NB1 + b
            a, e = seg_start1[kk], seg_end1[kk]
            n = e - a
            off = slot_off1[b]
            slots_src1[off:off + n] = t_src[a:e]
            coff = chunk_off1[b] * 128
            slots_pos1[coff:coff + n] = t_pos[a:e].astype(np.float32)
        idx1 = _wrap_idx(slots_src1.astype(np.int16))
        dl1 = np.ascontiguousarray(slots_pos1.reshape(NC1, 128).T)

        degloc = np.ones(NB0 * 128, np.float32)
        degloc[:NLOC] = deg0[local2global[c]]
        inv0 = np.ascontiguousarray(
            (1.0 / np.maximum(degloc, 1.0)).reshape(NB0, 128).T)

        deg1loc = np.ones(NBF * 128, np.float32)
        gl = SH1 * c + np.arange(SH1)
        valid = gl < N_DST1
        deg1loc[:SH1][valid] = deg1[gl[valid]]
        inv1 = np.ascontiguousarray(
            (1.0 / np.maximum(deg1loc, 1.0)).reshape(NBF, 128).T)

        fself = np.zeros((NB0 * 128, D_IN), np.float32)
        fself[:NLOC] = feat[local2global[c]]

        in_maps.append({
            "feat": feat, "fself": fself,
            "idx0": idx0, "dl0": dl0, "inv0": inv0,
            "idx1": idx1, "dl1": dl1, "inv1": inv1,
            "w0s": w0s, "w0n": w0n, "b0r": b0r,
            "w1s": w1s, "w1n": w1n, "b1r": b1r, "ii": ii,
        })

    meta = dict(Q0=Q0, CK0=CK0, slot_off0=slot_off0, chunk_off0=chunk_off0,
                TOT0=TOT0, NC0=NC0,
                Q1=Q1, CK1=CK1, slot_off1=slot_off1, chunk_off1=chunk_off1,
                TOT1=TOT1, NC1=NC1, s_core=s_core)
    return in_maps, meta


def _build_program(meta):
    import concourse.bass as bass
    import concourse.mybir as mybir
    import concourse.tile as tile
    from concourse import bacc

    f32 = mybir.dt.float32
    i16 = mybir.dt.int16
    Q0, CK0 = meta["Q0"], meta["CK0"]
    slot_off0, chunk_off0 = meta["slot_off0"], meta["chunk_off0"]
    TOT0, NC0 = meta["TOT0"], meta["NC0"]
    Q1, CK1 = meta["Q1"], meta["CK1"]
    slot_off1, chunk_off1 = meta["slot_off1"], meta["chunk_off1"]
    TOT1, NC1 = meta["TOT1"], meta["NC1"]

    nc = bacc.Bacc("TRN2", target_bir_lowering=False, debug=False,
                   enable_asserts=True, num_devices=C)
    t_feat = nc.dram_tensor("feat", [N_SRC0, D_IN], f32, kind="ExternalInput")
    t_fself = nc.dram_tensor("fself", [NB0 * 128, D_IN], f32, kind="ExternalInput")
    t_idx0 = nc.dram_tensor("idx0", [128, TOT0 // 16], i16, kind="ExternalInput")
    t_dl0 = nc.dram_tensor("dl0", [128, NC0], f32, kind="ExternalInput")
    t_inv0 = nc.dram_tensor("inv0", [128, NB0], f32, kind="ExternalInput")
    t_idx1 = nc.dram_tensor("idx1", [128, TOT1 // 16], i16, kind="ExternalInput")
    t_dl1 = nc.dram_tensor("dl1", [128, NC1], f32, kind="ExternalInput")
    t_inv1 = nc.dram_tensor("inv1", [128, NBF], f32, kind="ExternalInput")
    t_w0s = nc.dram_tensor("w0s", [128, 2 * D_HID], f32, kind="ExternalInput")
    t_w0n = nc.dram_tensor("w0n", [128, 2 * D_HID], f32, kind="ExternalInput")
    t_b0 = nc.dram_tensor("b0r", [1, D_HID], f32, kind="ExternalInput")
    t_w1s = nc.dram_tensor("w1s", [128, 4 * D_OUT], f32, kind="ExternalInput")
    t_w1n = nc.dram_tensor("w1n", [128, 4 * D_OUT], f32, kind="ExternalInput")
    t_b1 = nc.dram_tensor("b1r", [1, D_OUT], f32, kind="ExternalInput")
    t_ii = nc.dram_tensor("ii", [128, 256], f32, kind="ExternalInput")
    t_out = nc.dram_tensor("out", [SH1, D_OUT], f32, kind="ExternalOutput")
    t_hmy = nc.dram_tensor("hmy", [NB0 * 128, D_HID], f32)
    t_partial = nc.dram_tensor("partial", [PAD_DST1, D_HID], f32)
    t_rs = nc.dram_tensor("rsout", [SH1, D_HID], f32)

    eq = mybir.AluOpType.is_equal
    Relu = mybir.ActivationFunctionType.Relu

    with tile.TileContext(nc) as tc:
        with tc.tile_pool(name="const", bufs=1) as cp:
            idx0_t = cp.tile([128, TOT0 // 16], i16)
            nc.sync.dma_start(idx0_t[:], t_idx0[:, :])
            dl0_t = cp.tile([128, NC0], f32)
            nc.sync.dma_start(dl0_t[:], t_dl0[:, :])
            inv0_t = cp.tile([128, NB0], f32)
            nc.sync.dma_start(inv0_t[:], t_inv0[:, :])
            idx1_t = cp.tile([128, TOT1 // 16], i16)
            nc.sync.dma_start(idx1_t[:], t_idx1[:, :])
            dl1_t = cp.tile([128, NC1], f32)
            nc.sync.dma_start(dl1_t[:], t_dl1[:, :])
            inv1_t = cp.tile([128, NBF], f32)
            nc.sync.dma_start(inv1_t[:], t_inv1[:, :])
            w0s_t = cp.tile([128, 2 * D_HID], f32)
            nc.sync.dma_start(w0s_t[:], t_w0s[:, :])
            w0n_t = cp.tile([128, 2 * D_HID], f32)
            nc.sync.dma_start(w0n_t[:], t_w0n[:, :])
            b0_t = cp.tile([1, D_HID], f32)
            nc.sync.dma_start(b0_t[:], t_b0[:, :])
            w1s_t = cp.tile([128, 4 * D_OUT], f32)
            nc.sync.dma_start(w1s_t[:], t_w1s[:, :])
            w1n_t = cp.tile([128, 4 * D_OUT], f32)
            nc.sync.dma_start(w1n_t[:], t_w1n[:, :])
            b1_t = cp.tile([1, D_OUT], f32)
            nc.sync.dma_start(b1_t[:], t_b1[:, :])
            ii_t = cp.tile([128, 256], f32)
            nc.sync.dma_start(ii_t[:], t_ii[:, :])
            ones_t = cp.tile([1, 128], f32)
            nc.vector.memset(ones_t[:], 1.0)
            iota_t = ii_t[:, 0:128]
            ident_t = ii_t[:, 128:256]

            # ================= Layer 0 =================
            with tc.tile_pool(name="g0", bufs=4) as g0p, \
                 tc.tile_pool(name="s0", bufs=12) as s0p, \
                 tc.tile_pool(name="mm0", bufs=2) as mm0p, \
                 tc.tile_pool(name="xs0", bufs=2) as xs0p, \
                 tc.tile_pool(name="xt0", bufs=2) as xt0p, \
                 tc.tile_pool(name="h0", bufs=3) as h0p, \
                 tc.tile_pool(name="pmsg", bufs=4, space="PSUM") as pmsgp, \
                 tc.tile_pool(name="pout", bufs=2, space="PSUM") as poutp, \
                 tc.tile_pool(name="pt", bufs=2, space="PSUM") as ptp:
                ctx_l0 = nc.named_scope("L0"); ctx_l0.__enter__()
                for b in range(NB0):
                    nchunks = int(CK0[b].sum())
                    psum_msg = pmsgp.tile([128, D_IN], f32)
                    ci = 0
                    for k in range(NR0):
                        Q = int(Q0[b, k])
                        if Q == 0:
                            continue
                        CK = int(CK0[b, k])
                        rbase = k * RANGE
                        rsz = min(RANGE, N_SRC0 - rbase)
                        g = g0p.tile([128, CK, D_IN], f32, tag="g0")
                        c0 = int(slot_off0[b, k]) // 16
                        nc.gpsimd.dma_gather(
                            g[:, :, :], t_feat[rbase:rbase + rsz, :],
                            idx0_t[:, c0:c0 + Q // 16],
                            num_idxs=Q, num_idxs_reg=Q, elem_size=D_IN)
                        for j in range(CK):
                            q = int(chunk_off0[b, k]) + j
                            S = s0p.tile([128, 128], f32, tag="s0")
                            nc.vector.tensor_tensor(
                                out=S[:],
                                in0=dl0_t[:, q:q + 1].to_broadcast([128, 128]),
                                in1=iota_t, op=eq)
                            nc.tensor.matmul(
                                psum_msg[:], lhsT=S[:], rhs=g[:, j, :],
                                start=(ci == 0), stop=(ci == nchunks - 1))
                            ci += 1
                    msgm = mm0p.tile([128, D_IN], f32)
                    nc.vector.tensor_scalar_mul(
                        msgm[:], psum_msg[:], inv0_t[:, b:b + 1])
                    xs = xs0p.tile([128, D_IN], f32)
                    nc.sync.dma_start(xs[:], t_fself[b * 128:(b + 1) * 128, :])
                    xts = xt0p.tile([128, D_IN], f32, tag="xts")
                    xtm = xt0p.tile([128, D_IN], f32, tag="xtm")
                    for src_t, dst_t in ((xs, xts), (msgm, xtm)):
                        for kk in range(2):
                            pt = ptp.tile([128, 128], f32)
                            nc.tensor.transpose(
                                pt[:], src_t[:, kk * 128:(kk + 1) * 128],
                                ident_t)
                            nc.vector.tensor_copy(
                                dst_t[:, kk * 128:(kk + 1) * 128], pt[:])
                    pout = poutp.tile([128, D_HID], f32)
                    nc.tensor.matmul(pout[:], lhsT=xts[:, 0:128],
                                     rhs=w0s_t[:, 0:D_HID],
                                     start=True, stop=False)
                    nc.tensor.matmul(pout[:], lhsT=xts[:, 128:256],
                                     rhs=w0s_t[:, D_HID:2 * D_HID],
                                     start=False, stop=False)
                    nc.tensor.matmul(pout[:], lhsT=xtm[:, 0:128],
                                     rhs=w0n_t[:, 0:D_HID],
                                     start=False, stop=False)
                    nc.tensor.matmul(pout[:], lhsT=xtm[:, 128:256],
                                     rhs=w0n_t[:, D_HID:2 * D_HID],
                                     start=False, stop=False)
                    nc.tensor.matmul(pout[:], lhsT=ones_t[:1, :],
                                     rhs=b0_t[:1, :], start=False, stop=True)
                    h = h0p.tile([128, D_HID], f32)
                    nc.scalar.activation(h[:], pout[:], Relu)
                    nc.sync.dma_start(t_hmy[b * 128:(b + 1) * 128, :], h[:])
                ctx_l0.__exit__(None, None, None)

            # ================= Layer 1 partial sums =================
            with tc.tile_pool(name="g1", bufs=4) as g1p, \
                 tc.tile_pool(name="s1", bufs=12) as s1p, \
                 tc.tile_pool(name="pp1", bufs=3) as pp1p, \
                 tc.tile_pool(name="pm1", bufs=4, space="PSUM") as pm1p:
                ctx_l1 = nc.named_scope("L1"); ctx_l1.__enter__()
                for b in range(NB1):
                    Q = int(Q1[b])
                    if Q == 0:
                        continue
                    CK = int(CK1[b])
                    psum1 = pm1p.tile([128, D_HID], f32)
                    g = g1p.tile([128, CK, D_HID], f32, tag="g1")
                    c0 = int(slot_off1[b]) // 16
                    nc.gpsimd.dma_gather(
                        g[:, :, :], t_hmy[:, :], idx1_t[:, c0:c0 + Q // 16],
                        num_idxs=Q, num_idxs_reg=Q, elem_size=D_HID)
                    for j in range(CK):
                        q = int(chunk_off1[b]) + j
                        S = s1p.tile([128, 128], f32, tag="s1")
                        nc.vector.tensor_tensor(
                            out=S[:],
                            in0=dl1_t[:, q:q + 1].to_broadcast([128, 128]),
                            in1=iota_t, op=eq)
                        nc.tensor.matmul(
                            psum1[:], lhsT=S[:], rhs=g[:, j, :],
                            start=(j == 0), stop=(j == CK - 1))
                    part = pp1p.tile([128, D_HID], f32)
                    nc.vector.tensor_copy(part[:], psum1[:])
                    nc.sync.dma_start(
                        t_partial[b * 128:(b + 1) * 128, :], part[:])
                ctx_l1.__exit__(None, None, None)

            # ================= ReduceScatter =================
            nc.gpsimd.collective_compute(
                "ReduceScatter", mybir.AluOpType.add,
                replica_groups=[list(range(C))],
                ins=[t_partial.ap().opt()], outs=[t_rs.ap().opt()])

            # ================= Final layer-1 matmuls =================
            with tc.tile_pool(name="mf", bufs=2) as mfp, \
                 tc.tile_pool(name="hsf", bufs=2) as hsfp, \
                 tc.tile_pool(name="xtf", bufs=2) as xtfp, \
                 tc.tile_pool(name="of", bufs=2) as ofp, \
                 tc.tile_pool(name="pf", bufs=2, space="PSUM") as pfp, \
                 tc.tile_pool(name="ptf", bufs=2, space="PSUM") as ptfp:
                ctx_f = nc.named_scope("FIN"); ctx_f.__enter__()
                for fb in range(NBF):
                    nr = 128 if fb < NBF - 1 else SH1 - 128 * (NBF - 1)
                    m1 = mfp.tile([128, D_HID], f32, tag="m1")
                    nc.sync.dma_start(
                        m1[:nr, :], t_rs[fb * 128:fb * 128 + nr, :])
                    mm = mfp.tile([128, D_HID], f32, tag="mm")
                    nc.vector.tensor_scalar_mul(
                        mm[:nr, :], m1[:nr, :], inv1_t[:nr, fb:fb + 1])
                    hs = hsfp.tile([128, D_HID], f32)
                    nc.sync.dma_start(
                        hs[:nr, :], t_hmy[fb * 128:fb * 128 + nr, :])
                    xth = xtfp.tile([128, 4, 128], f32, tag="xth")
                    xtm1 = xtfp.tile([128, 4, 128], f32, tag="xtm1")
                    for src_t, dst_t in ((hs, xth), (mm, xtm1)):
                        for kk in range(4):
                            pt = ptfp.tile([128, 128], f32)
                            nc.tensor.transpose(
                                pt[:, :nr],
                                src_t[:nr, kk * 128:(kk + 1) * 128],
                                ident_t[:nr, :nr])
                            nc.vector.tensor_copy(
                                dst_t[:, kk, :nr], pt[:, :nr])
                    pf = pfp.tile([128, D_OUT], f32)
                    for kk in range(4):
                        nc.tensor.matmul(
                            pf[:nr, :], lhsT=xth[:, kk, :nr],
                            rhs=w1s_t[:, kk * D_OUT:(kk + 1) * D_OUT],
                            start=(kk == 0), stop=False)
                    for kk in range(4):
                        nc.tensor.matmul(
                            pf[:nr, :], lhsT=xtm1[:, kk, :nr],
                            rhs=w1n_t[:, kk * D_OUT:(kk + 1) * D_OUT],
                            start=False, stop=False)
                    nc.tensor.matmul(pf[:nr, :], lhsT=ones_t[:1, :nr],
                                     rhs=b1_t[:1, :], start=False, stop=True)
                    ot = ofp.tile([128, D_OUT], f32)
                    nc.vector.tensor_copy(ot[:nr, :], pf[:nr, :])
                    nc.sync.dma_start(
                        t_out[fb * 128:fb * 128 + nr, :], ot[:nr, :])
                ctx_f.__exit__(None, None, None)

    nc.compile()
    return nc


def _ensure_axon_hook():
    """Provide antenv.axon_hooks (missing in this image) so trace=True can
    capture NTFF profiles through the axon tunnel."""
    import types

    try:
        from antenv.axon_hooks import get_axon_ntff_profile_hook  # noqa: F401
        return
    except ImportError:
        pass
    mod = types.ModuleType("antenv.axon_hooks")
    _h = [None]
    mod.set_axon_ntff_profile_hook = lambda h: _h.__setitem__(0, h)
    mod.get_axon_ntff_profile_hook = lambda: _h[0]
    sys.modules["antenv.axon_hooks"] = mod
    import antenv

    antenv.axon_hooks = mod
    try:
        from trn_agent_boot.trn_boot import _ntff_profile_via_ctypes

        hook = _ntff_profile_via_ctypes("/opt/axon/libaxon_pjrt.so")
        if hook is not None:
            mod.set_axon_ntff_profile_hook(hook)
    except Exception:
        pass


def kernel(feat, Wself0, Wneigh0, b0, Wself1, Wneigh1, b1,
           edge_src0, edge_dst0, edge_src1, edge_dst1):
    global LAST_RESULT
    from concourse.bass_utils import run_bass_kernel_spmd

    if int(os.environ.get("KERNEL_TRACE", "0")):
        _ensure_axon_hook()

    in_maps, meta = _host_prep(feat, Wself0, Wneigh0, b0,
                               Wself1, Wneigh1, b1,
                               edge_src0, edge_dst0, edge_src1, edge_dst1)
    nc = _build_program(meta)
    res = run_bass_kernel_spmd(nc, in_maps, core_ids=list(range(C)),
                               trace=bool(int(os.environ.get("KERNEL_TRACE", "0"))))
    LAST_RESULT = res
    s_core = meta["s_core"]
    out = np.empty((N_DST1, D_OUT), np.float32)
    for c in range(C):
        out[SH1 * c:SH1 * c + s_core[c]] = res.results[c]["out"][:s_core[c]]
    return out
